# revision 6
# baseline (speedup 1.0000x reference)
"""Masked multi-head attention on 8 Trainium2 NeuronCores — bf16 dataflow.

Sharding: batch x head-group. Core c handles batch c//4 and heads
4*(c%4) .. 4*(c%4)+3 (Wq/Wk/Wv column-sharded, Wo row-sharded). Each core
computes a partial [S, D_MODEL] output = attn_heads @ Wo_slice; the host sums
the 4 partials per batch (the row-parallel reduce) and adds bo + bv @ Wo
(the bv term folds out because softmax rows sum to 1).

Optimizations over the f32r baseline:
  * all matmul operands + DMA'd tensors in bf16 (PSUM accumulation stays f32)
  * batched DMA: one descriptor per (input, s-block) / weight / store pair
  * exp for two heads fused into one activation over a 2-bank PSUM tile
  * flipped AV matmuls: et chunks stationary, v_aug moving (65 rows), so the
    PE streams ~2x fewer rows through attention; row sums land as a
    per-partition column, making softmax normalization a cheap
    tensor_scalar multiply (no partition_broadcast)
  * o transposed back to [dh, sq] via PE transposes for the out-projection
  * software pipelining: proj(j+1) / outproj matmuls emitted as fillers
    between attention tiles so the PE never starves while the scalar
    engine (exp) paces the attention inner loop
"""

import numpy as np

D_MODEL = 1024
N_HEAD = 16
HEAD_DIM = 64
B, S = 2, 2048
GH = 4  # heads per core
GC = GH * HEAD_DIM  # 256 dout columns per core
SBK = 512  # s block (moving free dim)
NSB = S // SBK  # 4 s blocks
NKT = D_MODEL // 128  # 8 din tiles
NST = S // 128  # 16 sk tiles

_CACHE = {}


def _build_nc():
    import concourse.mybir as mybir
    from concourse import bacc, tile

    F32 = mybir.dt.float32
    F32R = mybir.dt.float32r
    BF16 = mybir.dt.bfloat16
    EXP = mybir.ActivationFunctionType.Exp

    nc = bacc.Bacc(None, target_bir_lowering=False)

    xq = nc.declare_dram_parameter("xq", [D_MODEL, S], BF16, isOutput=False)
    xk = nc.declare_dram_parameter("xk", [D_MODEL, S], BF16, isOutput=False)
    xv = nc.declare_dram_parameter("xv", [D_MODEL, S], BF16, isOutput=False)
    wq = nc.declare_dram_parameter("wq", [D_MODEL, GC], BF16, isOutput=False)
    wk = nc.declare_dram_parameter("wk", [D_MODEL, GC], BF16, isOutput=False)
    wv = nc.declare_dram_parameter("wv", [D_MODEL, GC], BF16, isOutput=False)
    wo = nc.declare_dram_parameter("wo", [GC, D_MODEL], BF16, isOutput=False)
    # bias columns: [bq pt0, bq pt1, bk pt0, bk pt1]
    bias = nc.declare_dram_parameter("bias", [128, 4], F32, isOutput=False)
    y = nc.declare_dram_parameter("y", [S, D_MODEL], BF16, isOutput=True)

    xq_v = xq[:, :].rearrange("(k p) s -> p k s", p=128)
    xk_v = xk[:, :].rearrange("(k p) s -> p k s", p=128)
    xv_v = xv[:, :].rearrange("(k p) s -> p k s", p=128)
    wq_v = wq[:, :].rearrange("(k p) c -> p k c", p=128)
    wk_v = wk[:, :].rearrange("(k p) c -> p k c", p=128)
    wv_v = wv[:, :].rearrange("(k p) c -> p k c", p=128)
    wo_v = wo[:, :].rearrange("(t p) c -> p t c", p=128)
    y_v = y[:, :].rearrange("(t p) (e c) -> p t e c", p=128, c=SBK)

    with tile.TileContext(nc) as tc:
        with (
            tc.tile_pool(name="res", bufs=1) as res,
            tc.tile_pool(name="work", bufs=3) as work,
            tc.tile_pool(name="xin", bufs=2) as xin,
            tc.tile_pool(name="ps", bufs=2, space="PSUM") as ps,
        ):
            xviews = {"xq": xq_v, "xk": xk_v, "xv": xv_v}

            def load_x(nm, j, half=None):
                t = xin.tile([128, NKT, SBK], BF16, tag=nm, name=f"{nm}_{j}")
                if half is None:
                    nc.sync.dma_start(t[:], xviews[nm][:, :, j * SBK : (j + 1) * SBK])
                else:
                    for h0 in range(2):
                        nc.sync.dma_start(
                            t[:, h0 * 4 : (h0 + 1) * 4, :],
                            xviews[nm][:, h0 * 4 : (h0 + 1) * 4, j * SBK : (j + 1) * SBK],
                        )
                return t

            # ---- j=0 activations + weights: interleave the q/k/v paths in
            # half-tile DMAs so each projection path can start as soon as its
            # first operands land ----
            xq_t = {}
            xk_t = {}
            xv_t = {}
            wq_sb = res.tile([128, NKT, GC], BF16, tag="wq")
            wk_sb = res.tile([128, NKT, GC], BF16, tag="wk")
            wv_sb = res.tile([128, NKT, GC], BF16, tag="wv")
            bias_sb = res.tile([128, 4], F32, tag="bias")
            wo_sb = res.tile([128, 2, D_MODEL], BF16, tag="wo")
            xq_t[0] = xin.tile([128, NKT, SBK], BF16, tag="xq", name="xq_0")
            xk_t[0] = xin.tile([128, NKT, SBK], BF16, tag="xk", name="xk_0")
            xv_t[0] = xin.tile([128, NKT, SBK], BF16, tag="xv", name="xv_0")

            def half_load(t_sb, view, h0):
                nc.sync.dma_start(
                    t_sb[:, h0 * 4 : (h0 + 1) * 4, :],
                    view[:, h0 * 4 : (h0 + 1) * 4, 0:SBK],
                )

            nc.sync.dma_start(wq_sb[:], wq_v[:])
            half_load(xq_t[0], xq_v, 0)
            nc.sync.dma_start(bias_sb[:], bias[:, :])
            nc.sync.dma_start(wk_sb[:], wk_v[:])
            half_load(xq_t[0], xq_v, 1)
            half_load(xk_t[0], xk_v, 0)
            nc.sync.dma_start(wv_sb[:], wv_v[:])
            half_load(xk_t[0], xk_v, 1)
            half_load(xv_t[0], xv_v, 0)
            half_load(xv_t[0], xv_v, 1)
            nc.sync.dma_start(wo_sb[:], wo_v[:])

            # ---- identity [128, 128] for PE transposes ----
            ident = res.tile([128, 128], BF16, tag="ident")
            nc.gpsimd.memset(ident[:], 1.0)
            for pat, cm in (([[1, 128]], -1), ([[-1, 128]], 1)):
                nc.gpsimd.affine_select(
                    out=ident[:],
                    in_=ident[:],
                    compare_op=mybir.AluOpType.is_ge,
                    fill=0.0,
                    base=0,
                    pattern=pat,
                    channel_multiplier=cm,
                )

            # warm the PE clock ramp while the first DMAs are in flight
            # (scratch operand via a fast DVE memset so warming starts ~0.4us)
            warm_src = res.tile([128, 128], BF16, tag="warm_src")
            nc.vector.memset(warm_src[:], 1.0)
            warm = ps.tile([128, 2, SBK], F32, tag="sc", name="warm")
            for w in range(40):
                nc.tensor.matmul(
                    warm[:, 0, 0:128],
                    warm_src[:],
                    warm_src[:],
                    start=(w == 0),
                    stop=(w == 39),
                    skip_group_check=True,
                )
            # ---- causal triangle mask x2 [128, 2, 128]: keep y >= x ----
            maskt2 = res.tile([128, 2, 128], BF16, tag="maskt")
            nc.gpsimd.memset(maskt2[:], 1.0)
            for half in range(2):
                nc.gpsimd.affine_select(
                    out=maskt2[:, half, :],
                    in_=maskt2[:, half, :],
                    compare_op=mybir.AluOpType.is_ge,
                    fill=0.0,
                    base=0,
                    pattern=[[1, 128]],
                    channel_multiplier=-1,
                )
            # ---- resident activations ----
            qT_sb = [[res.tile([128, SBK], BF16, tag=f"qT_{pt}_{j}", name=f"qT_{pt}_{j}") for j in range(NSB)] for pt in range(2)]
            kT_sb = [[res.tile([128, SBK], BF16, tag=f"kT_{pt}_{j}", name=f"kT_{pt}_{j}") for j in range(NSB)] for pt in range(2)]
            # oT_sb[pt][j]: [128, 2, SBK]; partition = dh of head pair (po half),
            # free (a, c): sq tile t = a*2 + c//128
            oT_sb = [[res.tile([128, 2, 256], BF16, tag=f"oT_{pt}_{j}", name=f"oT_{pt}_{j}") for j in range(NSB)] for pt in range(2)]
            # v_aug[jb]: [128, 4(i in block), GH, 65]; cols 0..63 = v, col 64 = 1
            v_aug = [res.tile([128, 4, GH, HEAD_DIM + 1], BF16, tag=f"vaug_{jb}", name=f"vaug_{jb}") for jb in range(NSB)]
            ones_tmp = res.tile([128, 4, GH], BF16, tag="ones_tmp")
            nc.vector.memset(ones_tmp[:], 1.0)
            for jb in range(NSB):
                nc.vector.tensor_copy(v_aug[jb][:, :, :, HEAD_DIM], ones_tmp[:])

            qk_done = [False] * NSB

            def proj_units(j):
                """Yield once per projection matmul for block j (q, k, v)."""
                for which in range(2):  # 0 = q, 1 = k
                    w_sb, x_t, dst, bcol = (
                        (wq_sb, xq_t[j], qT_sb, 0) if which == 0 else (wk_sb, xk_t[j], kT_sb, 2)
                    )
                    for pt in range(2):
                        pq = ps.tile([128, SBK], F32, tag="proj")
                        for kt in range(NKT):
                            nc.tensor.matmul(
                                pq[:],
                                w_sb[:, kt, pt * 128 : (pt + 1) * 128],
                                x_t[:, kt, :],
                                start=(kt == 0),
                                stop=(kt == NKT - 1),
                            )
                            yield
                        nc.vector.tensor_scalar_add(
                            dst[pt][j][:], pq[:], bias_sb[:, bcol + pt : bcol + pt + 1]
                        )
                qk_done[j] = True
                for st in range(4):
                    pv = ps.tile([128, SBK], F32, tag="proj")
                    pvs = pv[:, :GC]
                    for kt in range(NKT):
                        nc.tensor.matmul(
                            pvs,
                            xv_t[j][:, kt, st * 128 : (st + 1) * 128],
                            wv_sb[:, kt],
                            start=(kt == 0),
                            stop=(kt == NKT - 1),
                        )
                        yield
                    pv3 = pvs.rearrange("p (h d) -> p h d", h=GH)
                    nc.vector.tensor_copy(v_aug[j][:, st, :, 0:HEAD_DIM], pv3[:])

            def outproj_units(j, act_copies=False):
                """Yield once per out-projection matmul for block j."""
                for tp in range(2):  # sq-tile pairs
                    y_sb = work.tile([128, 2, 2, SBK], BF16, tag="y_sb", bufs=4)
                    for tt in range(2):
                        t = tp * 2 + tt
                        for eb in range(2):
                            yp = ps.tile([128, SBK], F32, tag="proj")
                            for pt in range(2):
                                nc.tensor.matmul(
                                    yp[:],
                                    oT_sb[pt][j][:, t >> 1, (t & 1) * 128 : ((t & 1) + 1) * 128],
                                    wo_sb[:, pt, eb * SBK : (eb + 1) * SBK],
                                    start=(pt == 0),
                                    stop=(pt == 1),
                                )
                                yield
                            if not act_copies or (tt * 2 + eb) % 2 == 0:
                                nc.vector.tensor_copy(y_sb[:, tt, eb, :], yp[:])
                            else:
                                nc.scalar.activation(
                                    y_sb[:, tt, eb, :], yp[:],
                                    mybir.ActivationFunctionType.Copy,
                                )
                    t0 = j * 4 + tp * 2
                    nc.sync.dma_start(y_v[:, t0 : t0 + 2, :, :], y_sb[:])

            _DONE = object()

            def emit(filler, n):
                for _ in range(n):
                    if next(filler, _DONE) is _DONE:
                        return

            # ---- decoupled attention: a global "prep" stream (scores+exp+
            # mask, Act-paced) runs ahead of the AV/normalize stretches
            # (PE-paced), buffered by deep et tiles, so the scalar engine
            # works during PE-heavy projection bursts instead of pacing the
            # attention inner loop.
            stretch_order = [(j, hp) for j in range(NSB) for hp in range(2)]
            flat_tiles = [(j, hp, i) for (j, hp) in stretch_order for i in range(4 * (j + 1))]
            etq = {(j, hp): [] for (j, hp) in stretch_order}
            proj_done = [False] * NSB
            prep_pos = [0]

            def prep_tile(j, hp, i):
                pt = hp
                m = i - 4 * j
                c0 = 128 * m if m > 0 else 0
                sc2 = ps.tile([128, 2, SBK], F32, tag="sc")
                for half in range(2):
                    po = 64 * half
                    nc.tensor.matmul(
                        sc2[:, half, c0:],
                        kT_sb[pt][i // 4][po : po + 64, (i % 4) * 128 : (i % 4 + 1) * 128],
                        qT_sb[pt][j][po : po + 64, c0:],
                        start=True,
                        stop=True,
                    )
                et2 = work.tile([128, 2, SBK], BF16, tag="expt", bufs=24)
                nc.scalar.activation(et2[:, :, c0:], sc2[:, :, c0:], EXP, scale=0.125)
                if m >= 0:
                    nc.vector.tensor_mul(
                        et2[:, :, c0 : c0 + 128], et2[:, :, c0 : c0 + 128], maskt2[:]
                    )
                etq[(j, hp)].append(et2)

            def advance_prep(n):
                c = 0
                while c < n and prep_pos[0] < len(flat_tiles):
                    j, hp, i = flat_tiles[prep_pos[0]]
                    if not qk_done[j]:
                        return
                    prep_tile(j, hp, i)
                    prep_pos[0] += 1
                    c += 1

            CP = mybir.ActivationFunctionType.Copy

            def attn_stretch(j, hp, filler):
                last = (j == NSB - 1 and hp == 1)
                n_i = 4 * (j + 1)
                pt = hp
                # padded to a full 2KB PSUM bank; a matmul with start=True
                # zeroes the whole bank, so the t=3 chain starts first and
                # the other t chains accumulate onto the zeroed bank
                o_ps = [
                    ps.tile([128, 4, 128], F32, tag="av", name=f"o_ps{h}")
                    for h in range(2)
                ]
                advance_prep(4)
                q = etq[(j, hp)]
                for i in range(n_i):
                    m = i - 4 * j  # >= 0 on diagonal-straddling tiles
                    if len(q) <= i:
                        advance_prep(i + 1 - len(q))
                    et2 = q[i]
                    advance_prep(3)
                    emit(filler, 2)
                    for half in range(2):
                        ts = range(max(m, 0), 4)
                        if i == 0:
                            ts = (3, 2, 1, 0)
                        for t in ts:
                            nc.tensor.matmul(
                                o_ps[half][:, t, 0 : HEAD_DIM + 1],
                                et2[:, half, t * 128 : (t + 1) * 128],
                                v_aug[i // 4][:, i % 4, 2 * hp + half, :],
                                start=(i == 0 and t == 3),
                                stop=(m == t),
                                skip_group_check=True,
                            )
                q.clear()
                # normalize + transpose back to [dh, sq]
                oT_ps = ps.tile([128, 2, SBK], F32, tag="sc")
                oT_bf = oT_ps[:].bitcast(BF16)
                o_sbs = []
                with tc.high_priority(offset=64):
                    r4s = []
                    for half in range(2):
                        r4 = work.tile([128, 4], F32, tag="r4", bufs=4)
                        nc.vector.reciprocal(r4[:], o_ps[half][:, :, HEAD_DIM])
                        r4s.append(r4)
                        o_sb = work.tile(
                            [128, 4, HEAD_DIM], BF16, tag="o_sb", bufs=4,
                            name=f"o_sb{half}",
                        )
                        o_sbs.append(o_sb)
                    t_order = [2 * a + tb for a in range(2) for tb in range(2)] if last else range(4)
                    for t in t_order:
                        for half in range(2):
                            nc.vector.tensor_scalar_mul(
                                o_sbs[half][:, t, :],
                                o_ps[half][:, t, 0:HEAD_DIM],
                                r4s[half][:, t : t + 1],
                            )
                advance_prep(2)
                emit(filler, 4)
                for a in range(2):  # PSUM bank of oT_ps
                    seq = [(half, 2 * a + tb) for half in range(2) for tb in range(2)]
                    for idx, (half, t) in enumerate(seq):
                        po = 64 * half
                        nc.tensor.matmul(
                            oT_bf[po : po + 64, a, (t & 1) * 128 : ((t & 1) + 1) * 128],
                            o_sbs[half][:, t, :],
                            ident[:],
                            start=(idx == 0),
                            stop=(idx == 3),
                            is_transpose=True,
                            skip_group_check=True,
                        )
                    if last:
                        with tc.high_priority(offset=64):
                            nc.vector.tensor_copy(
                                oT_sb[pt][j][:, a, :], oT_bf[:, a, 0:256]
                            )
                if not last:
                    with tc.high_priority(offset=64):
                        nc.vector.tensor_copy(oT_sb[pt][j][:], oT_bf[:, :, 0:256])

            # ---- schedule ----
            fill_q = []

            def filler_stream():
                while True:
                    if not fill_q:
                        yield None
                        continue
                    g = fill_q[0]
                    v = next(g, _DONE)
                    if v is _DONE:
                        fill_q.pop(0)
                        continue
                    yield v

            filler = filler_stream()

            def emit(filler, n):
                for _ in range(n):
                    if not fill_q:
                        return
                    next(filler)

            def drain(gen):
                for _ in gen:
                    pass

            def proj_all(j):
                drain(proj_units(j))
                proj_done[j] = True

            def proj_filler(j):
                def g():
                    for u in proj_units(j):
                        yield u
                        if qk_done[j]:
                            advance_prep(1)
                    proj_done[j] = True
                return g()

            g0 = proj_units(0)
            while not qk_done[0]:
                next(g0)
            for _ in g0:
                advance_prep(1)
            proj_done[0] = True
            for jj in range(1, NSB):
                xq_t[jj] = load_x("xq", jj, half=True)
                xk_t[jj] = load_x("xk", jj, half=True)
                xv_t[jj] = load_x("xv", jj, half=True)
            advance_prep(6)
            attn_stretch(0, 0, filler)
            fill_q.append(proj_filler(1))
            attn_stretch(0, 1, filler)
            drain(filler_stream_until_empty()) if False else emit(filler, 10**9)
            advance_prep(8)
            fill_q.append(proj_filler(2))
            attn_stretch(1, 0, filler)
            attn_stretch(1, 1, filler)
            emit(filler, 10**9)
            advance_prep(8)
            fill_q.append(proj_filler(3))
            attn_stretch(2, 0, filler)
            fill_q.append(outproj_units(0))
            attn_stretch(2, 1, filler)
            emit(filler, 10**9)
            advance_prep(8)
            fill_q.append(outproj_units(1))
            attn_stretch(3, 0, filler)
            fill_q.append(outproj_units(2, act_copies=True))
            attn_stretch(3, 1, filler)
            emit(filler, 10**9)
            # final out-projection: copies alternate DVE/Act and stores go out
            # per sq tile so the drain only waits on the last small pieces
            for t in range(4):
                y_sb = work.tile([128, 1, 2, SBK], BF16, tag="y_sb3", bufs=4)
                t0 = (NSB - 1) * 4 + t
                yp2 = ps.tile([128, 2, SBK], F32, tag="sc")
                for eb in range(2):
                    for pt in range(2):
                        nc.tensor.matmul(
                            yp2[:, eb, :],
                            oT_sb[pt][NSB - 1][:, t >> 1, (t & 1) * 128 : ((t & 1) + 1) * 128],
                            wo_sb[:, pt, eb * SBK : (eb + 1) * SBK],
                            start=(pt == 0),
                            stop=(pt == 1),
                        )
                for eb in range(2):
                    if eb == 0:
                        nc.vector.tensor_copy(y_sb[:, 0, eb, :], yp2[:, eb, :])
                    else:
                        nc.scalar.activation(y_sb[:, 0, eb, :], yp2[:, eb, :], CP)
                    if t == 3:
                        nc.sync.dma_start(
                            y_v[:, t0 : t0 + 1, eb : eb + 1, :], y_sb[:, :, eb : eb + 1, :]
                        )
                if t < 3:
                    nc.sync.dma_start(y_v[:, t0 : t0 + 1, :, :], y_sb[:])

    nc.finalize()
    return nc


def _run_device(Q, K, V, Wq, bq, Wk, bk, Wv, Wo):
    import ml_dtypes
    from concourse.bass_utils import run_bass_kernel_spmd

    if "nc" not in _CACHE:
        _CACHE["nc"] = _build_nc()
    nc = _CACHE["nc"]

    bf16 = ml_dtypes.bfloat16
    in_maps = []
    xT = {}
    for b in range(B):
        xT[("q", b)] = np.ascontiguousarray(Q[b].T.astype(bf16))
        xT[("k", b)] = np.ascontiguousarray(K[b].T.astype(bf16))
        xT[("v", b)] = np.ascontiguousarray(V[b].T.astype(bf16))
    for c in range(8):
        b, g = c // 4, c % 4
        cs = slice(g * GC, (g + 1) * GC)
        bias_pack = np.stack(
            [bq[cs][:128], bq[cs][128:], bk[cs][:128], bk[cs][128:]], axis=1
        ).astype(np.float32)
        in_maps.append(
            {
                "xq": xT[("q", b)],
                "xk": xT[("k", b)],
                "xv": xT[("v", b)],
                "wq": np.ascontiguousarray(Wq[:, cs].astype(bf16)),
                "wk": np.ascontiguousarray(Wk[:, cs].astype(bf16)),
                "wv": np.ascontiguousarray(Wv[:, cs].astype(bf16)),
                "wo": np.ascontiguousarray(Wo[cs, :].astype(bf16)),
                "bias": np.ascontiguousarray(bias_pack),
            }
        )
    res = run_bass_kernel_spmd(nc, in_maps, core_ids=list(range(8)))
    return res


def kernel(Q, K, V, mask, Wq, bq, Wk, bk, Wv, bv, Wo, bo):
    Q = np.asarray(Q, dtype=np.float32)
    K = np.asarray(K, dtype=np.float32)
    V = np.asarray(V, dtype=np.float32)
    mask = np.asarray(mask)
    Wq, Wk, Wv, Wo = (np.asarray(a, dtype=np.float32) for a in (Wq, Wk, Wv, Wo))
    bq, bk, bv, bo = (np.asarray(a, dtype=np.float32) for a in (bq, bk, bv, bo))

    causal = bool(
        np.array_equal(mask[0], np.tril(np.ones((S, S), dtype=mask.dtype)))
    )
    if not causal:
        return _numpy_fallback(Q, K, V, mask, Wq, bq, Wk, bk, Wv, bv, Wo, bo)

    res = _run_device(Q, K, V, Wq, bq, Wk, bk, Wv, Wo)
    bo_eff = bo + bv @ Wo
    out = np.empty((B, S, D_MODEL), dtype=np.float32)
    for b in range(B):
        acc = res.results[4 * b]["y"].astype(np.float32)
        for g in range(1, 4):
            acc = acc + res.results[4 * b + g]["y"].astype(np.float32)
        out[b] = acc + bo_eff
    return out


def _numpy_fallback(Q, K, V, mask, Wq, bq, Wk, bk, Wv, bv, Wo, bo):
    out = np.empty((B, S, D_MODEL), dtype=np.float32)
    for b in range(B):
        q = (Q[b] @ Wq + bq).reshape(S, N_HEAD, HEAD_DIM).transpose(1, 0, 2)
        k = (K[b] @ Wk + bk).reshape(S, N_HEAD, HEAD_DIM).transpose(1, 0, 2)
        v = (V[b] @ Wv + bv).reshape(S, N_HEAD, HEAD_DIM).transpose(1, 0, 2)
        mb = mask[b] if mask.shape[0] > 1 else mask[0]
        o = np.empty((N_HEAD, S, HEAD_DIM), dtype=np.float32)
        for hh in range(N_HEAD):
            s = (q[hh] @ k[hh].T) / np.sqrt(np.float32(HEAD_DIM))
            s = np.where(mb == 0, -np.inf, s)
            s = s - s.max(-1, keepdims=True)
            e = np.exp(s)
            p = e / e.sum(-1, keepdims=True)
            o[hh] = p @ v[hh]
        out[b] = o.transpose(1, 0, 2).reshape(S, D_MODEL) @ Wo + bo
    return out


# revision 7
# speedup vs baseline: 1.0111x; 1.0111x over previous
"""Masked multi-head attention on 8 Trainium2 NeuronCores — bf16 dataflow.

Sharding: batch x head-group. Core c handles batch c//4 and heads
4*(c%4) .. 4*(c%4)+3 (Wq/Wk/Wv column-sharded, Wo row-sharded). Each core
computes a partial [S, D_MODEL] output = attn_heads @ Wo_slice; the host sums
the 4 partials per batch (the row-parallel reduce) and adds bo + bv @ Wo
(the bv term folds out because softmax rows sum to 1).

Optimizations over the f32r baseline:
  * all matmul operands + DMA'd tensors in bf16 (PSUM accumulation stays f32)
  * batched DMA: one descriptor per (input, s-block) / weight / store pair
  * exp for two heads fused into one activation over a 2-bank PSUM tile
  * flipped AV matmuls: et chunks stationary, v_aug moving (65 rows), so the
    PE streams ~2x fewer rows through attention; row sums land as a
    per-partition column, making softmax normalization a cheap
    tensor_scalar multiply (no partition_broadcast)
  * o transposed back to [dh, sq] via PE transposes for the out-projection
  * software pipelining: proj(j+1) / outproj matmuls emitted as fillers
    between attention tiles so the PE never starves while the scalar
    engine (exp) paces the attention inner loop
"""

import numpy as np

D_MODEL = 1024
N_HEAD = 16
HEAD_DIM = 64
B, S = 2, 2048
GH = 4  # heads per core
GC = GH * HEAD_DIM  # 256 dout columns per core
SBK = 512  # s block (moving free dim)
NSB = S // SBK  # 4 s blocks
NKT = D_MODEL // 128  # 8 din tiles
NST = S // 128  # 16 sk tiles

_CACHE = {}


def _build_nc():
    import concourse.mybir as mybir
    from concourse import bacc, tile

    F32 = mybir.dt.float32
    F32R = mybir.dt.float32r
    BF16 = mybir.dt.bfloat16
    EXP = mybir.ActivationFunctionType.Exp

    nc = bacc.Bacc(None, target_bir_lowering=False)

    xq = nc.declare_dram_parameter("xq", [D_MODEL, S], BF16, isOutput=False)
    xk = nc.declare_dram_parameter("xk", [D_MODEL, S], BF16, isOutput=False)
    xv = nc.declare_dram_parameter("xv", [D_MODEL, S], BF16, isOutput=False)
    wq = nc.declare_dram_parameter("wq", [D_MODEL, GC], BF16, isOutput=False)
    wk = nc.declare_dram_parameter("wk", [D_MODEL, GC], BF16, isOutput=False)
    wv = nc.declare_dram_parameter("wv", [D_MODEL, GC], BF16, isOutput=False)
    wo = nc.declare_dram_parameter("wo", [GC, D_MODEL], BF16, isOutput=False)
    # bias columns: [bq pt0, bq pt1, bk pt0, bk pt1]
    bias = nc.declare_dram_parameter("bias", [128, 4], F32, isOutput=False)
    y = nc.declare_dram_parameter("y", [S, D_MODEL], BF16, isOutput=True)

    xq_v = xq[:, :].rearrange("(k p) s -> p k s", p=128)
    xk_v = xk[:, :].rearrange("(k p) s -> p k s", p=128)
    xv_v = xv[:, :].rearrange("(k p) s -> p k s", p=128)
    wq_v = wq[:, :].rearrange("(k p) c -> p k c", p=128)
    wk_v = wk[:, :].rearrange("(k p) c -> p k c", p=128)
    wv_v = wv[:, :].rearrange("(k p) c -> p k c", p=128)
    wo_v = wo[:, :].rearrange("(t p) c -> p t c", p=128)
    y_v = y[:, :].rearrange("(t p) (e c) -> p t e c", p=128, c=SBK)

    with tile.TileContext(nc) as tc:
        with (
            tc.tile_pool(name="res", bufs=1) as res,
            tc.tile_pool(name="work", bufs=3) as work,
            tc.tile_pool(name="xin", bufs=2) as xin,
            tc.tile_pool(name="ps", bufs=2, space="PSUM") as ps,
        ):
            xviews = {"xq": xq_v, "xk": xk_v, "xv": xv_v}

            def load_x(nm, j, half=None):
                t = xin.tile([128, NKT, SBK], BF16, tag=nm, name=f"{nm}_{j}")
                if half is None:
                    nc.sync.dma_start(t[:], xviews[nm][:, :, j * SBK : (j + 1) * SBK])
                else:
                    for h0 in range(2):
                        nc.sync.dma_start(
                            t[:, h0 * 4 : (h0 + 1) * 4, :],
                            xviews[nm][:, h0 * 4 : (h0 + 1) * 4, j * SBK : (j + 1) * SBK],
                        )
                return t

            # ---- j=0 activations + weights: interleave the q/k/v paths in
            # half-tile DMAs so each projection path can start as soon as its
            # first operands land ----
            xq_t = {}
            xk_t = {}
            xv_t = {}
            wq_sb = res.tile([128, NKT, GC], BF16, tag="wq")
            wk_sb = res.tile([128, NKT, GC], BF16, tag="wk")
            wv_sb = res.tile([128, NKT, GC], BF16, tag="wv")
            bias_sb = res.tile([128, 4], F32, tag="bias")
            wo_sb = res.tile([128, 2, D_MODEL], BF16, tag="wo")
            xq_t[0] = xin.tile([128, NKT, SBK], BF16, tag="xq", name="xq_0")
            xk_t[0] = xin.tile([128, NKT, SBK], BF16, tag="xk", name="xk_0")
            xv_t[0] = xin.tile([128, NKT, SBK], BF16, tag="xv", name="xv_0")

            def half_load(t_sb, view, h0):
                nc.sync.dma_start(
                    t_sb[:, h0 * 4 : (h0 + 1) * 4, :],
                    view[:, h0 * 4 : (h0 + 1) * 4, 0:SBK],
                )

            nc.sync.dma_start(wq_sb[:], wq_v[:])
            half_load(xq_t[0], xq_v, 0)
            nc.sync.dma_start(bias_sb[:], bias[:, :])
            nc.sync.dma_start(wk_sb[:], wk_v[:])
            half_load(xq_t[0], xq_v, 1)
            half_load(xk_t[0], xk_v, 0)
            nc.sync.dma_start(wv_sb[:], wv_v[:])
            half_load(xk_t[0], xk_v, 1)
            half_load(xv_t[0], xv_v, 0)
            half_load(xv_t[0], xv_v, 1)
            nc.sync.dma_start(wo_sb[:], wo_v[:])

            # ---- identity [128, 128] for PE transposes ----
            ident = res.tile([128, 128], BF16, tag="ident")
            nc.gpsimd.memset(ident[:], 1.0)
            for pat, cm in (([[1, 128]], -1), ([[-1, 128]], 1)):
                nc.gpsimd.affine_select(
                    out=ident[:],
                    in_=ident[:],
                    compare_op=mybir.AluOpType.is_ge,
                    fill=0.0,
                    base=0,
                    pattern=pat,
                    channel_multiplier=cm,
                )

            # warm the PE clock ramp while the first DMAs are in flight
            # (scratch operand via a fast DVE memset so warming starts ~0.4us)
            warm_src = res.tile([128, 128], BF16, tag="warm_src")
            nc.vector.memset(warm_src[:], 1.0)
            warm = ps.tile([128, 2, SBK], F32, tag="sc", name="warm")
            for w in range(40):
                nc.tensor.matmul(
                    warm[:, 0, 0:128],
                    warm_src[:],
                    warm_src[:],
                    start=(w == 0),
                    stop=(w == 39),
                    skip_group_check=True,
                )
            # ---- causal triangle mask x2 [128, 2, 128]: keep y >= x ----
            maskt2 = res.tile([128, 2, 128], BF16, tag="maskt")
            nc.gpsimd.memset(maskt2[:], 1.0)
            for half in range(2):
                nc.gpsimd.affine_select(
                    out=maskt2[:, half, :],
                    in_=maskt2[:, half, :],
                    compare_op=mybir.AluOpType.is_ge,
                    fill=0.0,
                    base=0,
                    pattern=[[1, 128]],
                    channel_multiplier=-1,
                )
            # ---- resident activations ----
            qT_sb = [[res.tile([128, SBK], BF16, tag=f"qT_{pt}_{j}", name=f"qT_{pt}_{j}") for j in range(NSB)] for pt in range(2)]
            kT_sb = [[res.tile([128, SBK], BF16, tag=f"kT_{pt}_{j}", name=f"kT_{pt}_{j}") for j in range(NSB)] for pt in range(2)]
            # oT_sb[pt][j]: [128, 2, SBK]; partition = dh of head pair (po half),
            # free (a, c): sq tile t = a*2 + c//128
            oT_sb = [[res.tile([128, 2, 256], BF16, tag=f"oT_{pt}_{j}", name=f"oT_{pt}_{j}") for j in range(NSB)] for pt in range(2)]
            # v_aug[jb]: [128, 4(i in block), GH, 65]; cols 0..63 = v, col 64 = 1
            v_aug = [res.tile([128, 4, GH, HEAD_DIM + 1], BF16, tag=f"vaug_{jb}", name=f"vaug_{jb}") for jb in range(NSB)]
            ones_tmp = res.tile([128, 4, GH], BF16, tag="ones_tmp")
            nc.vector.memset(ones_tmp[:], 1.0)
            for jb in range(NSB):
                nc.vector.tensor_copy(v_aug[jb][:, :, :, HEAD_DIM], ones_tmp[:])

            qk_done = [False] * NSB

            def proj_units(j):
                """Yield once per projection matmul for block j (q, k, v)."""
                for which in range(2):  # 0 = q, 1 = k
                    w_sb, x_t, dst, bcol = (
                        (wq_sb, xq_t[j], qT_sb, 0) if which == 0 else (wk_sb, xk_t[j], kT_sb, 2)
                    )
                    for pt in range(2):
                        pq = ps.tile([128, SBK], F32, tag="proj")
                        for kt in range(NKT):
                            nc.tensor.matmul(
                                pq[:],
                                w_sb[:, kt, pt * 128 : (pt + 1) * 128],
                                x_t[:, kt, :],
                                start=(kt == 0),
                                stop=(kt == NKT - 1),
                            )
                            yield
                        nc.vector.tensor_scalar_add(
                            dst[pt][j][:], pq[:], bias_sb[:, bcol + pt : bcol + pt + 1]
                        )
                qk_done[j] = True
                for st in range(4):
                    pv = ps.tile([128, SBK], F32, tag="proj")
                    pvs = pv[:, :GC]
                    for kt in range(NKT):
                        nc.tensor.matmul(
                            pvs,
                            xv_t[j][:, kt, st * 128 : (st + 1) * 128],
                            wv_sb[:, kt],
                            start=(kt == 0),
                            stop=(kt == NKT - 1),
                        )
                        yield
                    pv3 = pvs.rearrange("p (h d) -> p h d", h=GH)
                    nc.vector.tensor_copy(v_aug[j][:, st, :, 0:HEAD_DIM], pv3[:])

            def outproj_units(j, act_copies=False):
                """Yield once per out-projection matmul for block j."""
                for tp in range(2):  # sq-tile pairs
                    y_sb = work.tile([128, 2, 2, SBK], BF16, tag="y_sb", bufs=4)
                    for tt in range(2):
                        t = tp * 2 + tt
                        for eb in range(2):
                            yp = ps.tile([128, SBK], F32, tag="proj")
                            for pt in range(2):
                                nc.tensor.matmul(
                                    yp[:],
                                    oT_sb[pt][j][:, t >> 1, (t & 1) * 128 : ((t & 1) + 1) * 128],
                                    wo_sb[:, pt, eb * SBK : (eb + 1) * SBK],
                                    start=(pt == 0),
                                    stop=(pt == 1),
                                )
                                yield
                            if not act_copies or (tt * 2 + eb) % 2 == 0:
                                nc.vector.tensor_copy(y_sb[:, tt, eb, :], yp[:])
                            else:
                                nc.scalar.activation(
                                    y_sb[:, tt, eb, :], yp[:],
                                    mybir.ActivationFunctionType.Copy,
                                )
                    t0 = j * 4 + tp * 2
                    nc.sync.dma_start(y_v[:, t0 : t0 + 2, :, :], y_sb[:])

            _DONE = object()

            def emit(filler, n):
                for _ in range(n):
                    if next(filler, _DONE) is _DONE:
                        return

            # ---- decoupled attention: a global "prep" stream (scores+exp+
            # mask, Act-paced) runs ahead of the AV/normalize stretches
            # (PE-paced), buffered by deep et tiles, so the scalar engine
            # works during PE-heavy projection bursts instead of pacing the
            # attention inner loop.
            stretch_order = [(j, hp) for j in range(NSB) for hp in range(2)]
            flat_tiles = [(j, hp, i) for (j, hp) in stretch_order for i in range(4 * (j + 1))]
            etq = {(j, hp): [] for (j, hp) in stretch_order}
            proj_done = [False] * NSB
            prep_pos = [0]

            def prep_tile(j, hp, i):
                pt = hp
                m = i - 4 * j
                c0 = 128 * m if m > 0 else 0
                sc2 = ps.tile([128, 2, SBK], F32, tag="sc")
                for half in range(2):
                    po = 64 * half
                    nc.tensor.matmul(
                        sc2[:, half, c0:],
                        kT_sb[pt][i // 4][po : po + 64, (i % 4) * 128 : (i % 4 + 1) * 128],
                        qT_sb[pt][j][po : po + 64, c0:],
                        start=True,
                        stop=True,
                    )
                et2 = work.tile([128, 2, SBK], BF16, tag="expt", bufs=24)
                nc.scalar.activation(et2[:, :, c0:], sc2[:, :, c0:], EXP, scale=0.125)
                if m >= 0:
                    nc.vector.tensor_mul(
                        et2[:, :, c0 : c0 + 128], et2[:, :, c0 : c0 + 128], maskt2[:]
                    )
                etq[(j, hp)].append(et2)

            def advance_prep(n):
                c = 0
                while c < n and prep_pos[0] < len(flat_tiles):
                    j, hp, i = flat_tiles[prep_pos[0]]
                    if not qk_done[j]:
                        return
                    prep_tile(j, hp, i)
                    prep_pos[0] += 1
                    c += 1

            CP = mybir.ActivationFunctionType.Copy

            def attn_stretch(j, hp, filler):
                last = (j == NSB - 1 and hp == 1)
                n_i = 4 * (j + 1)
                pt = hp
                # padded to a full 2KB PSUM bank; a matmul with start=True
                # zeroes the whole bank, so the t=3 chain starts first and
                # the other t chains accumulate onto the zeroed bank
                o_ps = [
                    ps.tile([128, 4, 128], F32, tag="av", name=f"o_ps{h}")
                    for h in range(2)
                ]
                advance_prep(4)
                q = etq[(j, hp)]
                for i in range(n_i):
                    m = i - 4 * j  # >= 0 on diagonal-straddling tiles
                    if len(q) <= i:
                        advance_prep(i + 1 - len(q))
                    et2 = q[i]
                    advance_prep(3)
                    emit(filler, 2)
                    for half in range(2):
                        ts = range(max(m, 0), 4)
                        if i == 0:
                            ts = (3, 2, 1, 0)
                        for t in ts:
                            nc.tensor.matmul(
                                o_ps[half][:, t, 0 : HEAD_DIM + 1],
                                et2[:, half, t * 128 : (t + 1) * 128],
                                v_aug[i // 4][:, i % 4, 2 * hp + half, :],
                                start=(i == 0 and t == 3),
                                stop=(m == t),
                                skip_group_check=True,
                            )
                q.clear()
                # normalize + transpose back to [dh, sq]
                oT_ps = ps.tile([128, 2, SBK], F32, tag="sc")
                oT_bf = oT_ps[:].bitcast(BF16)
                o_sbs = []
                with tc.high_priority(offset=64):
                    r4s = []
                    for half in range(2):
                        r4 = work.tile([128, 4], F32, tag="r4", bufs=4)
                        nc.vector.reciprocal(r4[:], o_ps[half][:, :, HEAD_DIM])
                        r4s.append(r4)
                        o_sb = work.tile(
                            [128, 4, HEAD_DIM], BF16, tag="o_sb", bufs=4,
                            name=f"o_sb{half}",
                        )
                        o_sbs.append(o_sb)
                    t_order = [2 * a + tb for a in range(2) for tb in range(2)] if last else range(4)
                    for t in t_order:
                        for half in range(2):
                            if last and half == 1:
                                nc.scalar.activation(
                                    o_sbs[half][:, t, :],
                                    o_ps[half][:, t, 0:HEAD_DIM],
                                    CP, scale=r4s[half][:, t : t + 1],
                                )
                            else:
                                nc.vector.tensor_scalar_mul(
                                    o_sbs[half][:, t, :],
                                    o_ps[half][:, t, 0:HEAD_DIM],
                                    r4s[half][:, t : t + 1],
                                )
                advance_prep(2)
                emit(filler, 16)
                for a in range(2):  # PSUM bank of oT_ps
                    seq = [(half, 2 * a + tb) for half in range(2) for tb in range(2)]
                    for idx, (half, t) in enumerate(seq):
                        po = 64 * half
                        nc.tensor.matmul(
                            oT_bf[po : po + 64, a, (t & 1) * 128 : ((t & 1) + 1) * 128],
                            o_sbs[half][:, t, :],
                            ident[:],
                            start=(idx == 0),
                            stop=(idx == 3),
                            is_transpose=True,
                            skip_group_check=True,
                        )
                    if last:
                        with tc.high_priority(offset=64):
                            nc.vector.tensor_copy(
                                oT_sb[pt][j][:, a, :], oT_bf[:, a, 0:256]
                            )
                if not last:
                    with tc.high_priority(offset=64):
                        nc.vector.tensor_copy(oT_sb[pt][j][:], oT_bf[:, :, 0:256])

            # ---- schedule ----
            fill_q = []

            def filler_stream():
                while True:
                    if not fill_q:
                        yield None
                        continue
                    g = fill_q[0]
                    v = next(g, _DONE)
                    if v is _DONE:
                        fill_q.pop(0)
                        continue
                    yield v

            filler = filler_stream()

            def emit(filler, n):
                for _ in range(n):
                    if not fill_q:
                        return
                    next(filler)

            def drain(gen):
                for _ in gen:
                    pass

            def proj_all(j):
                drain(proj_units(j))
                proj_done[j] = True

            def proj_filler(j):
                def g():
                    for u in proj_units(j):
                        yield u
                        if qk_done[j]:
                            advance_prep(1)
                    proj_done[j] = True
                return g()

            g0 = proj_units(0)
            while not qk_done[0]:
                next(g0)
            for _ in g0:
                advance_prep(1)
            proj_done[0] = True
            for jj in range(1, NSB):
                xq_t[jj] = load_x("xq", jj, half=True)
                xk_t[jj] = load_x("xk", jj, half=True)
                xv_t[jj] = load_x("xv", jj, half=True)
            advance_prep(6)
            attn_stretch(0, 0, filler)
            fill_q.append(proj_filler(1))
            attn_stretch(0, 1, filler)
            drain(filler_stream_until_empty()) if False else emit(filler, 10**9)
            advance_prep(8)
            fill_q.append(proj_filler(2))
            attn_stretch(1, 0, filler)
            attn_stretch(1, 1, filler)
            emit(filler, 10**9)
            advance_prep(8)
            fill_q.append(proj_filler(3))
            attn_stretch(2, 0, filler)
            fill_q.append(outproj_units(0))
            attn_stretch(2, 1, filler)
            emit(filler, 10**9)
            advance_prep(8)
            fill_q.append(outproj_units(1))
            attn_stretch(3, 0, filler)
            fill_q.append(outproj_units(2, act_copies=True))
            attn_stretch(3, 1, filler)
            emit(filler, 10**9)
            # final out-projection: copies alternate DVE/Act and stores go out
            # per sq tile so the drain only waits on the last small pieces
            for t in range(4):
                y_sb = work.tile([128, 1, 2, SBK], BF16, tag="y_sb3", bufs=4)
                t0 = (NSB - 1) * 4 + t
                yp2 = ps.tile([128, 2, SBK], F32, tag="sc")
                for eb in range(2):
                    for pt in range(2):
                        nc.tensor.matmul(
                            yp2[:, eb, :],
                            oT_sb[pt][NSB - 1][:, t >> 1, (t & 1) * 128 : ((t & 1) + 1) * 128],
                            wo_sb[:, pt, eb * SBK : (eb + 1) * SBK],
                            start=(pt == 0),
                            stop=(pt == 1),
                        )
                for eb in range(2):
                    if eb == 0:
                        nc.vector.tensor_copy(y_sb[:, 0, eb, :], yp2[:, eb, :])
                    else:
                        nc.scalar.activation(y_sb[:, 0, eb, :], yp2[:, eb, :], CP)
                nc.sync.dma_start(y_v[:, t0 : t0 + 1, :, :], y_sb[:])

    nc.finalize()
    return nc


def _run_device(Q, K, V, Wq, bq, Wk, bk, Wv, Wo):
    import ml_dtypes
    from concourse.bass_utils import run_bass_kernel_spmd

    if "nc" not in _CACHE:
        _CACHE["nc"] = _build_nc()
    nc = _CACHE["nc"]

    bf16 = ml_dtypes.bfloat16
    in_maps = []
    xT = {}
    for b in range(B):
        xT[("q", b)] = np.ascontiguousarray(Q[b].T.astype(bf16))
        xT[("k", b)] = np.ascontiguousarray(K[b].T.astype(bf16))
        xT[("v", b)] = np.ascontiguousarray(V[b].T.astype(bf16))
    for c in range(8):
        b, g = c // 4, c % 4
        cs = slice(g * GC, (g + 1) * GC)
        bias_pack = np.stack(
            [bq[cs][:128], bq[cs][128:], bk[cs][:128], bk[cs][128:]], axis=1
        ).astype(np.float32)
        in_maps.append(
            {
                "xq": xT[("q", b)],
                "xk": xT[("k", b)],
                "xv": xT[("v", b)],
                "wq": np.ascontiguousarray(Wq[:, cs].astype(bf16)),
                "wk": np.ascontiguousarray(Wk[:, cs].astype(bf16)),
                "wv": np.ascontiguousarray(Wv[:, cs].astype(bf16)),
                "wo": np.ascontiguousarray(Wo[cs, :].astype(bf16)),
                "bias": np.ascontiguousarray(bias_pack),
            }
        )
    res = run_bass_kernel_spmd(nc, in_maps, core_ids=list(range(8)))
    return res


def kernel(Q, K, V, mask, Wq, bq, Wk, bk, Wv, bv, Wo, bo):
    Q = np.asarray(Q, dtype=np.float32)
    K = np.asarray(K, dtype=np.float32)
    V = np.asarray(V, dtype=np.float32)
    mask = np.asarray(mask)
    Wq, Wk, Wv, Wo = (np.asarray(a, dtype=np.float32) for a in (Wq, Wk, Wv, Wo))
    bq, bk, bv, bo = (np.asarray(a, dtype=np.float32) for a in (bq, bk, bv, bo))

    causal = bool(
        np.array_equal(mask[0], np.tril(np.ones((S, S), dtype=mask.dtype)))
    )
    if not causal:
        return _numpy_fallback(Q, K, V, mask, Wq, bq, Wk, bk, Wv, bv, Wo, bo)

    res = _run_device(Q, K, V, Wq, bq, Wk, bk, Wv, Wo)
    bo_eff = bo + bv @ Wo
    out = np.empty((B, S, D_MODEL), dtype=np.float32)
    for b in range(B):
        acc = res.results[4 * b]["y"].astype(np.float32)
        for g in range(1, 4):
            acc = acc + res.results[4 * b + g]["y"].astype(np.float32)
        out[b] = acc + bo_eff
    return out


def _numpy_fallback(Q, K, V, mask, Wq, bq, Wk, bk, Wv, bv, Wo, bo):
    out = np.empty((B, S, D_MODEL), dtype=np.float32)
    for b in range(B):
        q = (Q[b] @ Wq + bq).reshape(S, N_HEAD, HEAD_DIM).transpose(1, 0, 2)
        k = (K[b] @ Wk + bk).reshape(S, N_HEAD, HEAD_DIM).transpose(1, 0, 2)
        v = (V[b] @ Wv + bv).reshape(S, N_HEAD, HEAD_DIM).transpose(1, 0, 2)
        mb = mask[b] if mask.shape[0] > 1 else mask[0]
        o = np.empty((N_HEAD, S, HEAD_DIM), dtype=np.float32)
        for hh in range(N_HEAD):
            s = (q[hh] @ k[hh].T) / np.sqrt(np.float32(HEAD_DIM))
            s = np.where(mb == 0, -np.inf, s)
            s = s - s.max(-1, keepdims=True)
            e = np.exp(s)
            p = e / e.sum(-1, keepdims=True)
            o[hh] = p @ v[hh]
        out[b] = o.transpose(1, 0, 2).reshape(S, D_MODEL) @ Wo + bo
    return out


# revision 9
# speedup vs baseline: 1.0115x; 1.0005x over previous
"""Masked multi-head attention on 8 Trainium2 NeuronCores — bf16 dataflow.

Sharding: batch x head-group. Core c handles batch c//4 and heads
4*(c%4) .. 4*(c%4)+3 (Wq/Wk/Wv column-sharded, Wo row-sharded). Each core
computes a partial [S, D_MODEL] output = attn_heads @ Wo_slice; the host sums
the 4 partials per batch (the row-parallel reduce) and adds bo + bv @ Wo
(the bv term folds out because softmax rows sum to 1).

Optimizations over the f32r baseline:
  * all matmul operands + DMA'd tensors in bf16 (PSUM accumulation stays f32)
  * batched DMA: one descriptor per (input, s-block) / weight / store pair
  * exp for two heads fused into one activation over a 2-bank PSUM tile
  * flipped AV matmuls: et chunks stationary, v_aug moving (65 rows), so the
    PE streams ~2x fewer rows through attention; row sums land as a
    per-partition column, making softmax normalization a cheap
    tensor_scalar multiply (no partition_broadcast)
  * o transposed back to [dh, sq] via PE transposes for the out-projection
  * software pipelining: proj(j+1) / outproj matmuls emitted as fillers
    between attention tiles so the PE never starves while the scalar
    engine (exp) paces the attention inner loop
"""

import numpy as np

D_MODEL = 1024
N_HEAD = 16
HEAD_DIM = 64
B, S = 2, 2048
GH = 4  # heads per core
GC = GH * HEAD_DIM  # 256 dout columns per core
SBK = 512  # s block (moving free dim)
NSB = S // SBK  # 4 s blocks
NKT = D_MODEL // 128  # 8 din tiles
NST = S // 128  # 16 sk tiles

_CACHE = {}


def _build_nc():
    import concourse.mybir as mybir
    from concourse import bacc, tile

    F32 = mybir.dt.float32
    F32R = mybir.dt.float32r
    BF16 = mybir.dt.bfloat16
    EXP = mybir.ActivationFunctionType.Exp

    nc = bacc.Bacc(None, target_bir_lowering=False)

    xq = nc.declare_dram_parameter("xq", [D_MODEL, S], BF16, isOutput=False)
    xk = nc.declare_dram_parameter("xk", [D_MODEL, S], BF16, isOutput=False)
    xv = nc.declare_dram_parameter("xv", [D_MODEL, S], BF16, isOutput=False)
    wq = nc.declare_dram_parameter("wq", [D_MODEL, GC], BF16, isOutput=False)
    wk = nc.declare_dram_parameter("wk", [D_MODEL, GC], BF16, isOutput=False)
    wv = nc.declare_dram_parameter("wv", [D_MODEL, GC], BF16, isOutput=False)
    wo = nc.declare_dram_parameter("wo", [GC, D_MODEL], BF16, isOutput=False)
    # bias columns: [bq pt0, bq pt1, bk pt0, bk pt1]
    bias = nc.declare_dram_parameter("bias", [128, 4], F32, isOutput=False)
    y = nc.declare_dram_parameter("y", [S, D_MODEL], BF16, isOutput=True)

    xq_v = xq[:, :].rearrange("(k p) s -> p k s", p=128)
    xk_v = xk[:, :].rearrange("(k p) s -> p k s", p=128)
    xv_v = xv[:, :].rearrange("(k p) s -> p k s", p=128)
    wq_v = wq[:, :].rearrange("(k p) c -> p k c", p=128)
    wk_v = wk[:, :].rearrange("(k p) c -> p k c", p=128)
    wv_v = wv[:, :].rearrange("(k p) c -> p k c", p=128)
    wo_v = wo[:, :].rearrange("(t p) c -> p t c", p=128)
    y_v = y[:, :].rearrange("(t p) (e c) -> p t e c", p=128, c=SBK)

    with tile.TileContext(nc) as tc:
        with (
            tc.tile_pool(name="res", bufs=1) as res,
            tc.tile_pool(name="work", bufs=3) as work,
            tc.tile_pool(name="xin", bufs=2) as xin,
            tc.tile_pool(name="ps", bufs=2, space="PSUM") as ps,
        ):
            xviews = {"xq": xq_v, "xk": xk_v, "xv": xv_v}

            def load_x(nm, j, half=None):
                t = xin.tile([128, NKT, SBK], BF16, tag=nm, name=f"{nm}_{j}")
                if half is None:
                    nc.sync.dma_start(t[:], xviews[nm][:, :, j * SBK : (j + 1) * SBK])
                else:
                    for h0 in range(2):
                        nc.sync.dma_start(
                            t[:, h0 * 4 : (h0 + 1) * 4, :],
                            xviews[nm][:, h0 * 4 : (h0 + 1) * 4, j * SBK : (j + 1) * SBK],
                        )
                return t

            # ---- j=0 activations + weights: interleave the q/k/v paths in
            # half-tile DMAs so each projection path can start as soon as its
            # first operands land ----
            xq_t = {}
            xk_t = {}
            xv_t = {}
            wq_sb = res.tile([128, NKT, GC], BF16, tag="wq")
            wk_sb = res.tile([128, NKT, GC], BF16, tag="wk")
            wv_sb = res.tile([128, NKT, GC], BF16, tag="wv")
            bias_sb = res.tile([128, 4], F32, tag="bias")
            wo_sb = res.tile([128, 2, D_MODEL], BF16, tag="wo")
            xq_t[0] = xin.tile([128, NKT, SBK], BF16, tag="xq", name="xq_0")
            xk_t[0] = xin.tile([128, NKT, SBK], BF16, tag="xk", name="xk_0")
            xv_t[0] = xin.tile([128, NKT, SBK], BF16, tag="xv", name="xv_0")

            def half_load(t_sb, view, h0):
                nc.sync.dma_start(
                    t_sb[:, h0 * 4 : (h0 + 1) * 4, :],
                    view[:, h0 * 4 : (h0 + 1) * 4, 0:SBK],
                )

            nc.sync.dma_start(wq_sb[:], wq_v[:])
            half_load(xq_t[0], xq_v, 0)
            half_load(xq_t[0], xq_v, 1)
            nc.sync.dma_start(bias_sb[:], bias[:, :])
            nc.sync.dma_start(wk_sb[:], wk_v[:])
            half_load(xk_t[0], xk_v, 0)
            nc.sync.dma_start(wv_sb[:], wv_v[:])
            half_load(xk_t[0], xk_v, 1)
            half_load(xv_t[0], xv_v, 0)
            half_load(xv_t[0], xv_v, 1)
            nc.sync.dma_start(wo_sb[:], wo_v[:])

            # ---- identity [128, 128] for PE transposes ----
            ident = res.tile([128, 128], BF16, tag="ident")
            nc.gpsimd.memset(ident[:], 1.0)
            for pat, cm in (([[1, 128]], -1), ([[-1, 128]], 1)):
                nc.gpsimd.affine_select(
                    out=ident[:],
                    in_=ident[:],
                    compare_op=mybir.AluOpType.is_ge,
                    fill=0.0,
                    base=0,
                    pattern=pat,
                    channel_multiplier=cm,
                )

            # warm the PE clock ramp while the first DMAs are in flight
            # (scratch operand via a fast DVE memset so warming starts ~0.4us)
            warm_src = res.tile([128, 128], BF16, tag="warm_src")
            nc.vector.memset(warm_src[:], 1.0)
            warm = ps.tile([128, 2, SBK], F32, tag="sc", name="warm")
            for w in range(40):
                nc.tensor.matmul(
                    warm[:, 0, 0:128],
                    warm_src[:],
                    warm_src[:],
                    start=(w == 0),
                    stop=(w == 39),
                    skip_group_check=True,
                )
            # ---- causal triangle mask x2 [128, 2, 128]: keep y >= x ----
            maskt2 = res.tile([128, 2, 128], BF16, tag="maskt")
            nc.gpsimd.memset(maskt2[:], 1.0)
            for half in range(2):
                nc.gpsimd.affine_select(
                    out=maskt2[:, half, :],
                    in_=maskt2[:, half, :],
                    compare_op=mybir.AluOpType.is_ge,
                    fill=0.0,
                    base=0,
                    pattern=[[1, 128]],
                    channel_multiplier=-1,
                )
            # ---- resident activations ----
            qT_sb = [[res.tile([128, SBK], BF16, tag=f"qT_{pt}_{j}", name=f"qT_{pt}_{j}") for j in range(NSB)] for pt in range(2)]
            kT_sb = [[res.tile([128, SBK], BF16, tag=f"kT_{pt}_{j}", name=f"kT_{pt}_{j}") for j in range(NSB)] for pt in range(2)]
            # oT_sb[pt][j]: [128, 2, SBK]; partition = dh of head pair (po half),
            # free (a, c): sq tile t = a*2 + c//128
            oT_sb = [[res.tile([128, 2, 256], BF16, tag=f"oT_{pt}_{j}", name=f"oT_{pt}_{j}") for j in range(NSB)] for pt in range(2)]
            # v_aug[jb]: [128, 4(i in block), GH, 65]; cols 0..63 = v, col 64 = 1
            v_aug = [res.tile([128, 4, GH, HEAD_DIM + 1], BF16, tag=f"vaug_{jb}", name=f"vaug_{jb}") for jb in range(NSB)]
            ones_tmp = res.tile([128, 4, GH], BF16, tag="ones_tmp")
            nc.vector.memset(ones_tmp[:], 1.0)
            for jb in range(NSB):
                nc.vector.tensor_copy(v_aug[jb][:, :, :, HEAD_DIM], ones_tmp[:])

            qk_done = [False] * NSB

            def proj_units(j):
                """Yield once per projection matmul for block j (q, k, v)."""
                for which in range(2):  # 0 = q, 1 = k
                    w_sb, x_t, dst, bcol = (
                        (wq_sb, xq_t[j], qT_sb, 0) if which == 0 else (wk_sb, xk_t[j], kT_sb, 2)
                    )
                    for pt in range(2):
                        pq = ps.tile([128, SBK], F32, tag="proj")
                        for kt in range(NKT):
                            nc.tensor.matmul(
                                pq[:],
                                w_sb[:, kt, pt * 128 : (pt + 1) * 128],
                                x_t[:, kt, :],
                                start=(kt == 0),
                                stop=(kt == NKT - 1),
                            )
                            yield
                        nc.vector.tensor_scalar_add(
                            dst[pt][j][:], pq[:], bias_sb[:, bcol + pt : bcol + pt + 1]
                        )
                qk_done[j] = True
                for st in range(4):
                    pv = ps.tile([128, SBK], F32, tag="proj")
                    pvs = pv[:, :GC]
                    for kt in range(NKT):
                        nc.tensor.matmul(
                            pvs,
                            xv_t[j][:, kt, st * 128 : (st + 1) * 128],
                            wv_sb[:, kt],
                            start=(kt == 0),
                            stop=(kt == NKT - 1),
                        )
                        yield
                    pv3 = pvs.rearrange("p (h d) -> p h d", h=GH)
                    nc.vector.tensor_copy(v_aug[j][:, st, :, 0:HEAD_DIM], pv3[:])

            def outproj_units(j, act_copies=False):
                """Yield once per out-projection matmul for block j."""
                for tp in range(2):  # sq-tile pairs
                    y_sb = work.tile([128, 2, 2, SBK], BF16, tag="y_sb", bufs=4)
                    for tt in range(2):
                        t = tp * 2 + tt
                        for eb in range(2):
                            yp = ps.tile([128, SBK], F32, tag="proj")
                            for pt in range(2):
                                nc.tensor.matmul(
                                    yp[:],
                                    oT_sb[pt][j][:, t >> 1, (t & 1) * 128 : ((t & 1) + 1) * 128],
                                    wo_sb[:, pt, eb * SBK : (eb + 1) * SBK],
                                    start=(pt == 0),
                                    stop=(pt == 1),
                                )
                                yield
                            if not act_copies or (tt * 2 + eb) % 2 == 0:
                                nc.vector.tensor_copy(y_sb[:, tt, eb, :], yp[:])
                            else:
                                nc.scalar.activation(
                                    y_sb[:, tt, eb, :], yp[:],
                                    mybir.ActivationFunctionType.Copy,
                                )
                    t0 = j * 4 + tp * 2
                    nc.sync.dma_start(y_v[:, t0 : t0 + 2, :, :], y_sb[:])

            _DONE = object()

            def emit(filler, n):
                for _ in range(n):
                    if next(filler, _DONE) is _DONE:
                        return

            # ---- decoupled attention: a global "prep" stream (scores+exp+
            # mask, Act-paced) runs ahead of the AV/normalize stretches
            # (PE-paced), buffered by deep et tiles, so the scalar engine
            # works during PE-heavy projection bursts instead of pacing the
            # attention inner loop.
            stretch_order = [(j, hp) for j in range(NSB) for hp in range(2)]
            flat_tiles = [(j, hp, i) for (j, hp) in stretch_order for i in range(4 * (j + 1))]
            etq = {(j, hp): [] for (j, hp) in stretch_order}
            proj_done = [False] * NSB
            prep_pos = [0]

            def prep_tile(j, hp, i):
                pt = hp
                m = i - 4 * j
                c0 = 128 * m if m > 0 else 0
                sc2 = ps.tile([128, 2, SBK], F32, tag="sc")
                for half in range(2):
                    po = 64 * half
                    nc.tensor.matmul(
                        sc2[:, half, c0:],
                        kT_sb[pt][i // 4][po : po + 64, (i % 4) * 128 : (i % 4 + 1) * 128],
                        qT_sb[pt][j][po : po + 64, c0:],
                        start=True,
                        stop=True,
                    )
                et2 = work.tile([128, 2, SBK], BF16, tag="expt", bufs=28)
                nc.scalar.activation(et2[:, :, c0:], sc2[:, :, c0:], EXP, scale=0.125)
                if m >= 0:
                    nc.vector.tensor_mul(
                        et2[:, :, c0 : c0 + 128], et2[:, :, c0 : c0 + 128], maskt2[:]
                    )
                etq[(j, hp)].append(et2)

            def advance_prep(n):
                c = 0
                while c < n and prep_pos[0] < len(flat_tiles):
                    j, hp, i = flat_tiles[prep_pos[0]]
                    if not qk_done[j]:
                        return
                    prep_tile(j, hp, i)
                    prep_pos[0] += 1
                    c += 1

            CP = mybir.ActivationFunctionType.Copy

            def attn_stretch(j, hp, filler):
                last = (j == NSB - 1 and hp == 1)
                n_i = 4 * (j + 1)
                pt = hp
                # padded to a full 2KB PSUM bank; a matmul with start=True
                # zeroes the whole bank, so the t=3 chain starts first and
                # the other t chains accumulate onto the zeroed bank
                o_ps = [
                    ps.tile([128, 4, 128], F32, tag="av", name=f"o_ps{h}")
                    for h in range(2)
                ]
                advance_prep(4)
                q = etq[(j, hp)]
                for i in range(n_i):
                    m = i - 4 * j  # >= 0 on diagonal-straddling tiles
                    if len(q) <= i:
                        advance_prep(i + 1 - len(q))
                    et2 = q[i]
                    advance_prep(3)
                    emit(filler, 2)
                    for half in range(2):
                        ts = range(max(m, 0), 4)
                        if i == 0:
                            ts = (3, 2, 1, 0)
                        for t in ts:
                            nc.tensor.matmul(
                                o_ps[half][:, t, 0 : HEAD_DIM + 1],
                                et2[:, half, t * 128 : (t + 1) * 128],
                                v_aug[i // 4][:, i % 4, 2 * hp + half, :],
                                start=(i == 0 and t == 3),
                                stop=(m == t),
                                skip_group_check=True,
                            )
                q.clear()
                # normalize + transpose back to [dh, sq]
                oT_ps = ps.tile([128, 2, SBK], F32, tag="sc")
                oT_bf = oT_ps[:].bitcast(BF16)
                o_sbs = []
                with tc.high_priority(offset=64):
                    r4s = []
                    for half in range(2):
                        r4 = work.tile([128, 4], F32, tag="r4", bufs=4)
                        nc.vector.reciprocal(r4[:], o_ps[half][:, :, HEAD_DIM])
                        r4s.append(r4)
                        o_sb = work.tile(
                            [128, 4, HEAD_DIM], BF16, tag="o_sb", bufs=4,
                            name=f"o_sb{half}",
                        )
                        o_sbs.append(o_sb)
                    def norm_mul(t, half):
                        if last and half == 1:
                            nc.scalar.activation(
                                o_sbs[half][:, t, :],
                                o_ps[half][:, t, 0:HEAD_DIM],
                                CP, scale=r4s[half][:, t : t + 1],
                            )
                        else:
                            nc.vector.tensor_scalar_mul(
                                o_sbs[half][:, t, :],
                                o_ps[half][:, t, 0:HEAD_DIM],
                                r4s[half][:, t : t + 1],
                            )

                    for t in range(4):
                        for half in range(2):
                            norm_mul(t, half)
                advance_prep(2)
                emit(filler, 16)
                for a in range(2):  # PSUM bank of oT_ps
                    seq = [(half, 2 * a + tb) for half in range(2) for tb in range(2)]
                    for idx, (half, t) in enumerate(seq):
                        po = 64 * half
                        nc.tensor.matmul(
                            oT_bf[po : po + 64, a, (t & 1) * 128 : ((t & 1) + 1) * 128],
                            o_sbs[half][:, t, :],
                            ident[:],
                            start=(idx == 0),
                            stop=(idx == 3),
                            is_transpose=True,
                            skip_group_check=True,
                        )
                    if last:
                        with tc.high_priority(offset=64):
                            nc.vector.tensor_copy(
                                oT_sb[pt][j][:, a, :], oT_bf[:, a, 0:256]
                            )
                if not last:
                    with tc.high_priority(offset=64):
                        nc.vector.tensor_copy(oT_sb[pt][j][:], oT_bf[:, :, 0:256])

            # ---- schedule ----
            fill_q = []

            def filler_stream():
                while True:
                    if not fill_q:
                        yield None
                        continue
                    g = fill_q[0]
                    v = next(g, _DONE)
                    if v is _DONE:
                        fill_q.pop(0)
                        continue
                    yield v

            filler = filler_stream()

            def emit(filler, n):
                for _ in range(n):
                    if not fill_q:
                        return
                    next(filler)

            def drain(gen):
                for _ in gen:
                    pass

            def proj_all(j):
                drain(proj_units(j))
                proj_done[j] = True

            def proj_filler(j):
                def g():
                    for u in proj_units(j):
                        yield u
                        if qk_done[j]:
                            advance_prep(1)
                    proj_done[j] = True
                return g()

            g0 = proj_units(0)
            while not qk_done[0]:
                next(g0)
            for _ in g0:
                advance_prep(1)
            proj_done[0] = True
            for jj in range(1, NSB):
                xq_t[jj] = load_x("xq", jj, half=True)
                xk_t[jj] = load_x("xk", jj, half=True)
                xv_t[jj] = load_x("xv", jj, half=True)
            advance_prep(6)
            attn_stretch(0, 0, filler)
            fill_q.append(proj_filler(1))
            attn_stretch(0, 1, filler)
            drain(filler_stream_until_empty()) if False else emit(filler, 10**9)
            advance_prep(8)
            fill_q.append(proj_filler(2))
            attn_stretch(1, 0, filler)
            attn_stretch(1, 1, filler)
            emit(filler, 10**9)
            advance_prep(8)
            fill_q.append(proj_filler(3))
            attn_stretch(2, 0, filler)
            fill_q.append(outproj_units(0))
            attn_stretch(2, 1, filler)
            emit(filler, 10**9)
            advance_prep(8)
            fill_q.append(outproj_units(1))
            attn_stretch(3, 0, filler)
            fill_q.append(outproj_units(2, act_copies=True))
            attn_stretch(3, 1, filler)
            emit(filler, 10**9)
            # final out-projection: copies alternate DVE/Act and stores go out
            # per sq tile so the drain only waits on the last small pieces
            for t in range(4):
                y_sb = work.tile([128, 1, 2, SBK], BF16, tag="y_sb3", bufs=4)
                t0 = (NSB - 1) * 4 + t
                yp2 = ps.tile([128, 2, SBK], F32, tag="sc")
                for eb in range(2):
                    for pt in range(2):
                        nc.tensor.matmul(
                            yp2[:, eb, :],
                            oT_sb[pt][NSB - 1][:, t >> 1, (t & 1) * 128 : ((t & 1) + 1) * 128],
                            wo_sb[:, pt, eb * SBK : (eb + 1) * SBK],
                            start=(pt == 0),
                            stop=(pt == 1),
                        )
                for eb in range(2):
                    if eb == 0:
                        nc.vector.tensor_copy(y_sb[:, 0, eb, :], yp2[:, eb, :])
                    else:
                        nc.scalar.activation(y_sb[:, 0, eb, :], yp2[:, eb, :], CP)
                nc.sync.dma_start(y_v[:, t0 : t0 + 1, :, :], y_sb[:])

    nc.finalize()
    return nc


def _run_device(Q, K, V, Wq, bq, Wk, bk, Wv, Wo):
    import ml_dtypes
    from concourse.bass_utils import run_bass_kernel_spmd

    if "nc" not in _CACHE:
        _CACHE["nc"] = _build_nc()
    nc = _CACHE["nc"]

    bf16 = ml_dtypes.bfloat16
    in_maps = []
    xT = {}
    for b in range(B):
        xT[("q", b)] = np.ascontiguousarray(Q[b].T.astype(bf16))
        xT[("k", b)] = np.ascontiguousarray(K[b].T.astype(bf16))
        xT[("v", b)] = np.ascontiguousarray(V[b].T.astype(bf16))
    for c in range(8):
        b, g = c // 4, c % 4
        cs = slice(g * GC, (g + 1) * GC)
        bias_pack = np.stack(
            [bq[cs][:128], bq[cs][128:], bk[cs][:128], bk[cs][128:]], axis=1
        ).astype(np.float32)
        in_maps.append(
            {
                "xq": xT[("q", b)],
                "xk": xT[("k", b)],
                "xv": xT[("v", b)],
                "wq": np.ascontiguousarray(Wq[:, cs].astype(bf16)),
                "wk": np.ascontiguousarray(Wk[:, cs].astype(bf16)),
                "wv": np.ascontiguousarray(Wv[:, cs].astype(bf16)),
                "wo": np.ascontiguousarray(Wo[cs, :].astype(bf16)),
                "bias": np.ascontiguousarray(bias_pack),
            }
        )
    res = run_bass_kernel_spmd(nc, in_maps, core_ids=list(range(8)))
    return res


def kernel(Q, K, V, mask, Wq, bq, Wk, bk, Wv, bv, Wo, bo):
    Q = np.asarray(Q, dtype=np.float32)
    K = np.asarray(K, dtype=np.float32)
    V = np.asarray(V, dtype=np.float32)
    mask = np.asarray(mask)
    Wq, Wk, Wv, Wo = (np.asarray(a, dtype=np.float32) for a in (Wq, Wk, Wv, Wo))
    bq, bk, bv, bo = (np.asarray(a, dtype=np.float32) for a in (bq, bk, bv, bo))

    causal = bool(
        np.array_equal(mask[0], np.tril(np.ones((S, S), dtype=mask.dtype)))
    )
    if not causal:
        return _numpy_fallback(Q, K, V, mask, Wq, bq, Wk, bk, Wv, bv, Wo, bo)

    res = _run_device(Q, K, V, Wq, bq, Wk, bk, Wv, Wo)
    bo_eff = bo + bv @ Wo
    out = np.empty((B, S, D_MODEL), dtype=np.float32)
    for b in range(B):
        acc = res.results[4 * b]["y"].astype(np.float32)
        for g in range(1, 4):
            acc = acc + res.results[4 * b + g]["y"].astype(np.float32)
        out[b] = acc + bo_eff
    return out


def _numpy_fallback(Q, K, V, mask, Wq, bq, Wk, bk, Wv, bv, Wo, bo):
    out = np.empty((B, S, D_MODEL), dtype=np.float32)
    for b in range(B):
        q = (Q[b] @ Wq + bq).reshape(S, N_HEAD, HEAD_DIM).transpose(1, 0, 2)
        k = (K[b] @ Wk + bk).reshape(S, N_HEAD, HEAD_DIM).transpose(1, 0, 2)
        v = (V[b] @ Wv + bv).reshape(S, N_HEAD, HEAD_DIM).transpose(1, 0, 2)
        mb = mask[b] if mask.shape[0] > 1 else mask[0]
        o = np.empty((N_HEAD, S, HEAD_DIM), dtype=np.float32)
        for hh in range(N_HEAD):
            s = (q[hh] @ k[hh].T) / np.sqrt(np.float32(HEAD_DIM))
            s = np.where(mb == 0, -np.inf, s)
            s = s - s.max(-1, keepdims=True)
            e = np.exp(s)
            p = e / e.sum(-1, keepdims=True)
            o[hh] = p @ v[hh]
        out[b] = o.transpose(1, 0, 2).reshape(S, D_MODEL) @ Wo + bo
    return out
